# revision 31
# baseline (speedup 1.0000x reference)
"""Trainium2 Bass kernel for nn_AttentiveQuantizer (vq_codebook).

Reference computation (per pixel p of n*h*w, cin=cout=192, K=1024):
    q = latent_p @ wq.T ; k = codebook @ wk.T ; v = codebook @ wv.T
    logit_p = q @ k.T / sqrt(192) * temperature1            -> output 3
    trueCode_p = argmax(logit_p)                            -> output 2
    z_p = logit_p + gumbel(key=42)
    idx_p = argmax(z_p)   (softmax is monotone; hard straight-through
                           sample is exactly one-hot with hot value
                           (1+y)-y = 1 +- 2e-7, so)
    quantized_p ~= v[idx_p]                                 -> output 1

Device algorithm (per core = one batch element n, 4096 pixels):
    logit = xT.T @ WT         with WT = (wq.T @ wk @ codebook.T ... folded
                              on host in float64: ((codebook@wk.T)@wq).T * scale)
    z = logit + gumbel        (DVE tensor_tensor_reduce, fused row-max)
    idx = max_index(z)        (DVE)
    trueCode = max_index(logit) with row max from a GPSIMD max-tree
    quantized rows = indirect-DMA gather of v[idx]

Sharding: data-parallel over batch n (8 cores, one n each).
"""

import os
import numpy as np

import concourse.bacc as bacc
import concourse.bass as bass
import concourse.mybir as mybir
import concourse.tile as tile
from concourse.bass_utils import run_bass_kernel_spmd

N, CIN, COUT, H, W, K = 8, 192, 192, 64, 64, 1024
PIX = H * W          # pixels per core
P = 128              # partitions / pixels per tile
F32 = mybir.dt.float32
U32 = mybir.dt.uint32
I32 = mybir.dt.int32
I16 = mybir.dt.int16

# Engine notes (hardware-verified):
# - ANT custom DVE ops (tensor_tensor_reduce, ...) wedge the device under this
#   axon terminal -> native InstMax / InstMaxIndex / InstTensorTensor only.
# - Pool (gpsimd) tensor_tensor supports add (not max) in walrus codegen.

# Candidate-set pruning of the gumbel argmax: the gumbel tensor is a known
# constant and |logit[p,k]| <= A[p,k] (rank-R SVD triangle bound, fp64-safe),
# so argmax(logit+g) provably lies in {k : g[p,k] + A[p,k] >= max_j(g[p,j] -
# A[p,j])}.  Per 16-pixel Q7 group we ship the candidate union (<= C16 of
# 1024), scan only those, and map the winner back through the candidate list.
# Cuts the gumbel DMA ~4x and the z-side DVE scans ~4x in the cost model, but
# measured SLOWER on hardware (the ap_gather -> add -> scan -> map-back chain
# serializes through the Q7s) -> disabled; the full-scan kernel is the default.
USE_CAND = False
C16 = 256        # candidate-union capacity per 16-pixel group
SVD_R = 32       # rank of the bound's SVD head
CAND_MARGIN = 1e-3
PAD_NEG = np.float32(-3.0e38)


def _fold_weights(codebook, wq, wk, wv, temperature1):
    cb = codebook.astype(np.float64)
    ktab = cb @ wk.astype(np.float64).T                       # [K, COUT]
    scale = float(temperature1) / np.sqrt(np.float64(CIN))
    wfull = (ktab @ wq.astype(np.float64)) * scale            # [K, CIN]
    wt = np.ascontiguousarray(wfull.T).astype(np.float32)     # [CIN, K]
    vtab = np.ascontiguousarray(cb @ wv.astype(np.float64).T).astype(np.float32)
    return wt, vtab


def _gumbel_noise():
    """Reproduce the reference's jax.random.gumbel(key(42), [N,H,W,K], f32).

    Pinned to the CPU backend: the reference runs under JAX_PLATFORMS=cpu and
    rbg random bits are backend-dependent.
    """
    import jax

    cpu = jax.devices("cpu")[0]
    with jax.default_device(cpu):
        g = jax.random.gumbel(jax.random.key(42), (N, H, W, K), dtype="float32")
    return np.ascontiguousarray(np.asarray(g).reshape(N, PIX, K))


def build_candidates(x_all, wt, gum_all, c16=C16, r=SVD_R):
    """Exact candidate sets for argmax(logit + gumbel), per 16-pixel Q7 group.

    x_all: [n_pix_total, CIN] f32 pixels, gum_all: [n_pix_total, K] f32.
    Returns (cidx16 [n_pix, c16//16] i16 wrapped for ap_gather,
             gcand [n_pix, c16] f32 (pad -3e38),
             lgflat [n_groups*c16] i32 map-back table,
             gbase [n_pix, 1] u32 = group_id * c16).

    Bound: |logit[p,k]| = |x_p . W_k| <= sum_i s_i|<x,u_i>||v_ik| +
    ||x_perp||*||Wk_perp|| computed in fp64/fp32 with CAND_MARGIN slack; any k
    outside the candidate set satisfies z_k < max z strictly (also under fp32
    evaluation noise ~1e-5 << margin).
    """
    n_pix = x_all.shape[0]
    assert n_pix % P == 0
    xd = x_all.astype(np.float64)
    wd = wt.astype(np.float64)  # [CIN, K]
    u, s, vt = np.linalg.svd(wd, full_matrices=False)
    ur, sr, vr = u[:, :r], s[:r], vt[:r]
    proj = np.abs(xd @ ur).astype(np.float32)                # [n_pix, r]
    sv = (sr[:, None] * np.abs(vr)).astype(np.float32)       # [r, K]
    xperp = np.sqrt(
        np.maximum(np.einsum("ij,ij->i", xd, xd) - (proj.astype(np.float64) ** 2).sum(1), 0)
    ).astype(np.float32)
    wperp = np.sqrt(
        np.maximum((wd**2).sum(0) - ((sr[:, None] * vr) ** 2).sum(0), 0)
    ).astype(np.float32)
    A = proj @ sv + np.outer(xperp, wperp) + np.float32(CAND_MARGIN)
    lo = (gum_all - A).max(axis=1)                           # [n_pix]
    mask = (gum_all + A) >= lo[:, None]

    pid = np.arange(n_pix)
    gid = (pid // P) * 8 + (pid % P) // 16                   # group per pixel
    n_groups = n_pix // 16
    rows, cols = np.nonzero(mask)
    keys = np.unique(gid[rows].astype(np.int64) * K + cols)
    g_of_key = (keys // K).astype(np.int64)
    k_of_key = (keys % K).astype(np.int64)
    counts = np.bincount(g_of_key, minlength=n_groups)
    if counts.max() > c16:
        return None  # caller falls back to the full-scan kernel
    starts = np.zeros(n_groups + 1, np.int64)
    np.cumsum(counts, out=starts[1:])
    pos = np.arange(len(keys)) - starts[g_of_key]            # slot within group

    lg = np.zeros((n_groups, c16), np.int32)                 # pad 0
    lg[g_of_key, pos] = k_of_key
    valid = np.zeros((n_groups, c16), bool)
    valid[g_of_key, pos] = True

    # gcand[p, i] = g[p, lg[gid(p), i]] where valid else PAD_NEG
    lg_p = lg[gid]                                           # [n_pix, c16]
    gcand = np.take_along_axis(gum_all, lg_p, axis=1)
    gcand[~valid[gid]] = PAD_NEG

    # ap_gather wrapped indices: per tile of 128 partitions, core c uses
    # partitions 16c..16c+15; index slot n -> partition 16c + n%16, col n//16
    cidx16 = np.zeros((n_pix, c16 // 16), np.int16)
    lg16 = lg.astype(np.int16)                               # pad index 0
    slot = np.arange(c16)
    part16 = slot % 16                                       # target partition within group
    col16 = slot // 16
    for t in range(n_pix // P):
        for c in range(8):
            gi = t * 8 + c
            cidx16[t * P + 16 * c + part16, col16] = lg16[gi]

    gbase = (gid.astype(np.uint32) * np.uint32(c16)).reshape(n_pix, 1)
    return cidx16, gcand, lg.reshape(-1).astype(np.int32), gbase


def build_nc(n_tiles=PIX // P, *, use_cand=False, ab=frozenset(), bufs=None):
    """Build the per-core Bass program (SPMD; every core runs the same NEFF).

    ab: set of feature-ablation flags for cost-model A/B timing experiments
        ("no_scans", "no_add", "no_gather", "no_logit_out").
    """
    pix = n_tiles * P
    n_groups = pix // 16
    bufs = {**{"psum": 2, "gum": 3, "zbuf": 2, "lsb": 3, "qrow": 3, "small": 4},
            **(bufs or {})}
    nc = bacc.Bacc("TRN2", target_bir_lowering=False, debug=False)

    xt_d = nc.dram_tensor("xt", [CIN, pix], F32, kind="ExternalInput")
    if use_cand:
        # cidx/gbase pre-transposed on host to [P, n_tiles * _] for one-shot load
        cidx_d = nc.dram_tensor("cidx", [P, n_tiles * (C16 // 16)], I16, kind="ExternalInput")
        gcand_d = nc.dram_tensor("gcand", [pix, C16], F32, kind="ExternalInput")
        lgflat_d = nc.dram_tensor("lgflat", [n_groups * C16, 1], I32, kind="ExternalInput")
        gbase_d = nc.dram_tensor("gbase", [P, n_tiles], U32, kind="ExternalInput")
    else:
        gum_d = nc.dram_tensor("gum", [pix, K], F32, kind="ExternalInput")
    wt_d = nc.dram_tensor("wt", [CIN, K], F32, kind="ExternalInput")
    vtab_d = nc.dram_tensor("vtab", [K, COUT], F32, kind="ExternalInput")
    logit_d = nc.dram_tensor("logit_out", [pix, K], F32, kind="ExternalOutput")
    quant_d = nc.dram_tensor("quant_out", [pix, COUT], F32, kind="ExternalOutput")
    tcode_d = nc.dram_tensor("tcode_out", [pix], I32, kind="ExternalOutput")

    with tile.TileContext(nc) as tc:
        with (
            tc.tile_pool(name="consts", bufs=1) as consts,
            tc.tile_pool(name="psum", bufs=bufs["psum"], space="PSUM") as psum_p,
            tc.tile_pool(name="gum", bufs=bufs["gum"]) as gum_p,
            tc.tile_pool(name="zbuf", bufs=bufs["zbuf"]) as z_p,
            tc.tile_pool(name="lsb", bufs=bufs["lsb"]) as lsb_p,
            tc.tile_pool(name="qrow", bufs=bufs["qrow"]) as qrow_p,
            tc.tile_pool(name="small", bufs=bufs["small"]) as small_p,
        ):
            # --- persistent constants ---
            wt_hi = consts.tile([P, K], F32)
            wt_lo = consts.tile([CIN - P, K], F32)
            nc.sync.dma_start(wt_hi[:], wt_d[0:P, :])
            nc.sync.dma_start(wt_lo[:], wt_d[P:CIN, :])

            # x in independent column-chunk tiles so the first matmuls only
            # wait on chunk 0
            xchunk = min(512, pix)
            x_hi_chunks, x_lo_chunks = [], []
            for ci, c0 in enumerate(range(0, pix, xchunk)):
                c1 = c0 + xchunk
                xh = consts.tile([P, xchunk], F32, tag=f"xh{ci}")
                xl = consts.tile([CIN - P, xchunk], F32, tag=f"xl{ci}")
                nc.sync.dma_start(xh[:], xt_d[0:P, c0:c1])
                nc.sync.dma_start(xl[:], xt_d[P:CIN, c0:c1])
                x_hi_chunks.append(xh)
                x_lo_chunks.append(xl)

            # trueCode accumulator: col block t*8 gets tile t's max_index output
            tc8 = consts.tile([P, n_tiles * 8], U32)

            if use_cand:
                cidx_all = consts.tile([P, n_tiles * (C16 // 16)], I16)
                gbase_all = consts.tile([P, n_tiles], U32)
                nc.sync.dma_start(cidx_all[:], cidx_d[:])
                nc.sync.dma_start(gbase_all[:], gbase_d[:])

            tiles_per_chunk = xchunk // P
            for t in range(n_tiles):
                r0, r1 = t * P, (t + 1) * P
                xh = x_hi_chunks[t // tiles_per_chunk]
                xl = x_lo_chunks[t // tiles_per_chunk]
                q0 = (t % tiles_per_chunk) * P
                q1 = q0 + P

                # logit tile via PE (fp32): [128 pixels, 1024 codes]
                psum = psum_p.tile([P, K], F32, tag="psum")
                for h in range(2):
                    cs = slice(h * 512, (h + 1) * 512)
                    nc.tensor.matmul(
                        psum[:, cs], xh[:, q0:q1], wt_hi[:, cs],
                        start=True, stop=False,
                    )
                    nc.tensor.matmul(
                        psum[:, cs], xl[:, q0:q1], wt_lo[:, cs],
                        start=False, stop=True,
                    )

                # logit PSUM -> SBUF (ACT), for DMA-out + trueCode scan + z
                lsb = lsb_p.tile([P, K], F32, tag="lsb")
                for h in range(2):
                    cs = slice(h * 512, (h + 1) * 512)
                    nc.scalar.copy(lsb[:, cs], psum[:, cs])

                if use_cand:
                    # per-pixel gumbel at candidate codes
                    gcand = gum_p.tile([P, C16], F32, tag="gcand")
                    nc.sync.dma_start(gcand[:], gcand_d[r0:r1, :])

                    # logit at this group's candidate codes (per-core shared idx)
                    nci = C16 // 16
                    lcand = z_p.tile([P, C16], F32, tag="lcand")
                    nc.gpsimd.ap_gather(
                        lcand[:], lsb[:], cidx_all[:, t * nci : (t + 1) * nci],
                        channels=P, num_elems=K, d=1, num_idxs=C16,
                    )
                    zc = z_p.tile([P, C16], F32, tag="zc")
                    nc.vector.tensor_tensor(
                        out=zc[:], in0=lcand[:], in1=gcand[:],
                        op=mybir.AluOpType.add,
                    )
                    mz8 = small_p.tile([P, 8], F32, tag="mz8")
                    c8 = small_p.tile([P, 8], U32, tag="c8")
                    nc.vector.max(mz8[:], zc[:])
                    nc.vector.max_index(c8[:], mz8[:], zc[:])
                    # map back: k* = lgflat[gbase + c*]
                    offs = small_p.tile([P, 1], U32, tag="offs")
                    nc.vector.tensor_tensor(
                        out=offs[:], in0=gbase_all[:, t : t + 1], in1=c8[:, 0:1],
                        op=mybir.AluOpType.add,
                    )
                    kstar = small_p.tile([P, 1], I32, tag="kstar")
                    nc.gpsimd.indirect_dma_start(
                        out=kstar[:],
                        out_offset=None,
                        in_=lgflat_d[:],
                        in_offset=bass.IndirectOffsetOnAxis(ap=offs[:, 0:1], axis=0),
                    )
                    idx_for_gather = kstar
                else:
                    # gumbel tile in
                    gum = gum_p.tile([P, K], F32, tag="gum")
                    if "no_gum_in" not in ab:
                        nc.sync.dma_start(gum[:], gum_d[r0:r1, :])
                    else:
                        gum = lsb  # timing ablation: reuse logit as "gumbel"

                    # z = logit + gumbel (gpsimd; frees the DVE for the scans)
                    z = z_p.tile([P, K], F32, tag="z")
                    idxz8 = small_p.tile([P, 8], U32, tag="idxz8")
                    if "no_add" not in ab:
                        nc.gpsimd.tensor_tensor(
                            out=z[:], in0=lsb[:], in1=gum[:], op=mybir.AluOpType.add
                        )
                    else:
                        z = gum  # timing ablation: scan the raw gumbel
                    if "no_scans" not in ab:
                        mz8 = small_p.tile([P, 8], F32, tag="mz8")
                        nc.vector.max(mz8[:], z[:])
                        nc.vector.max_index(idxz8[:], mz8[:], z[:])
                    else:
                        nc.vector.memset(idxz8[:], 0)
                    idx_for_gather = idxz8

                if "no_scans" not in ab:
                    # trueCode = argmax(logit) (DVE scans)
                    ml8 = small_p.tile([P, 8], F32, tag="ml8")
                    nc.vector.max(ml8[:], lsb[:])
                    nc.vector.max_index(
                        tc8[:, t * 8 : (t + 1) * 8], ml8[:], lsb[:]
                    )
                else:
                    nc.vector.memset(tc8[:, t * 8 : (t + 1) * 8], 0)

                if "no_gather" not in ab:
                    # quantized rows = vtab[idx]  (indirect gather, gpsimd SWDGE)
                    qrow = qrow_p.tile([P, COUT], F32, tag="qrow")
                    nc.gpsimd.indirect_dma_start(
                        out=qrow[:],
                        out_offset=None,
                        in_=vtab_d[:],
                        in_offset=bass.IndirectOffsetOnAxis(
                            ap=idx_for_gather[:, 0:1], axis=0
                        ),
                    )
                    nc.scalar.dma_start(quant_d[r0:r1, :], qrow[:])

                # outputs
                if "no_logit_out" not in ab:
                    nc.sync.dma_start(logit_d[r0:r1, :], lsb[:])

            # trueCode out: tc8[:, t*8] is pixel (t*128 + p)'s argmax
            tc_src = tc8[:].rearrange("p (t e) -> p t e", e=8)[:, :, 0:1]
            tc_dst = tcode_d[:].rearrange("(t p) -> p t", p=P)
            nc.sync.dma_start(tc_dst, tc_src.bitcast(I32))

    nc.compile()
    return nc


_CACHE = {}


def _get_nc(n_tiles, use_cand=False):
    key = (n_tiles, use_cand)
    if key not in _CACHE:
        _CACHE[key] = build_nc(n_tiles, use_cand=use_cand)
    return _CACHE[key]


def kernel(latent, codebook, wq, wk, wv, temperature1, temperature, **_unused):
    latent = np.ascontiguousarray(np.asarray(latent, dtype=np.float32))
    wt, vtab = _fold_weights(
        np.asarray(codebook), np.asarray(wq), np.asarray(wk), np.asarray(wv),
        np.asarray(temperature1, dtype=np.float32),
    )
    gum = _gumbel_noise()

    xt = latent.reshape(N, CIN, PIX)

    cand = None
    if USE_CAND:
        x_all = np.transpose(xt, (0, 2, 1)).reshape(N * PIX, CIN)
        cand = build_candidates(x_all, wt, gum.reshape(N * PIX, K))

    if cand is not None:
        cidx16, gcand, lgflat, gbase = cand
        nt = PIX // P
        nci = C16 // 16
        cidx16 = cidx16.reshape(N, nt, P, nci).transpose(0, 2, 1, 3).reshape(
            N, P, nt * nci
        )
        gcand = gcand.reshape(N, PIX, C16)
        lgflat = lgflat.reshape(N, PIX // 16 * C16, 1)
        # gbase carries global group ids; each core's lgflat is local
        gbase = gbase.reshape(N, PIX) - (
            np.arange(N, dtype=np.uint32) * np.uint32(PIX // 16 * C16)
        ).reshape(N, 1)
        gbase = gbase.reshape(N, nt, P).transpose(0, 2, 1)  # [N, P, nt]
        in_maps = [
            {
                "xt": np.ascontiguousarray(xt[c]),
                "cidx": np.ascontiguousarray(cidx16[c]),
                "gcand": np.ascontiguousarray(gcand[c]),
                "lgflat": np.ascontiguousarray(lgflat[c]),
                "gbase": np.ascontiguousarray(gbase[c]),
                "wt": wt,
                "vtab": vtab,
            }
            for c in range(N)
        ]
        nc = _get_nc(PIX // P, use_cand=True)
    else:
        in_maps = [
            {
                "xt": np.ascontiguousarray(xt[c]),
                "gum": gum[c],
                "wt": wt,
                "vtab": vtab,
            }
            for c in range(N)
        ]
        nc = _get_nc(PIX // P, use_cand=False)

    res = run_bass_kernel_spmd(nc, in_maps, core_ids=list(range(N)))
    outs = res.results

    logit = np.stack([outs[c]["logit_out"] for c in range(N)]).reshape(N, H, W, K)
    tcode = np.stack([outs[c]["tcode_out"] for c in range(N)]).reshape(N, H, W)
    quant = np.stack([outs[c]["quant_out"] for c in range(N)])
    quantized = np.ascontiguousarray(
        quant.reshape(N, H, W, COUT).transpose(0, 3, 1, 2)
    )
    return quantized, tcode.astype(np.int32), logit
